# revision 8
# baseline (speedup 1.0000x reference)
"""CenterLoss kernel for 8 Trainium2 NeuronCores (Bass/Tile).

Problem (hardcoded, from nn_CenterLoss):
    h      [262144, 256] f32
    d      [262144]      int   (class ids in [0, 100000))
    center [100000, 256] f32
    returns (loss, new_center):
        loss       = mean((h - center[d])**2)                       scalar f32
        new_center = center + 0.1 * where(count>0, segmean(h)-center, 0)

Sharding strategy: shard along n_class. Core c owns classes
[c*12500, (c+1)*12500) and the matching center rows; the host routes each
batch row to the core owning its class (sorted by class id). Segment sums
then never need a cross-device reduction.

Device algorithm per core:
  - Classes are processed in 98 tiles of 128. For class tile j the batch
    rows belonging to it (padded with zero rows to n_j*128, n_j static and
    identical on every core so the SPMD program is uniform) are multiplied
    by an on-device-built one-hot selector and accumulated into PSUM:
        sums[128, 256] += onehot[128b, 128c].T @ h[128b, 256]
    The selector for all n_j sub-tiles is built with ONE broadcast
    is_equal compare against a resident iota tile (host pre-shifts the
    class ids by -128j so the compare needs no per-tile offset).
  - loss uses the expansion sum|h|^2 - 2*sum_k s_k.c_k + sum_k n_k|c_k|^2
    so center[d] is never gathered. The dot products are fused
    multiply+free-dim-reduce ops (scalar_tensor_tensor accum_out / ACT
    Square accum_out) over wide group tiles.
  - new_center tile = ainv*sums + oma*c in one fused stt op per tile,
    with ainv = alpha*present/count, oma = 1 - alpha*present per class.
  - All DRAM streams are laid out partition-major on the host so each
    multi-tile group moves with a single fully-contiguous DMA.
"""

import numpy as np

BATCH = 262144
F = 256
C = 100000
ALPHA = 0.1
N_CORES = 8
P = 128
C_SHARD = C // N_CORES          # 12500 classes per core
CT = (C_SHARD + P - 1) // P     # 98 class tiles per core
C_PAD = CT * P                  # 12544 rows incl. padding

H_GROUP_COLS = 12               # max batch sub-tiles per h DMA group
C_GROUP = 4                     # class tiles per center/PSUM group
ACT_HSQ_MOD = 2                 # h-group idx % MOD == 0 -> |h|^2 on ACT

_prog_cache = {}


def _h_groups(n_j):
    """Greedily group consecutive class tiles so each h DMA moves at most
    H_GROUP_COLS 128-row sub-tiles."""
    groups = []
    cur = []
    cols = 0
    for j, nj in enumerate(n_j):
        if cur and cols + nj > H_GROUP_COLS:
            groups.append(cur)
            cur, cols = [], 0
        cur.append(j)
        cols += nj
    if cur:
        groups.append(cur)
    return groups


def _c_groups():
    return [list(range(g, min(g + C_GROUP, CT))) for g in range(0, CT, C_GROUP)]


def _shard_inputs(h, d, center):
    """Route batch rows to the core owning their class; build per-core
    partition-major blocks so every device DMA is contiguous.

    Returns (in_maps, n_j)."""
    h = np.ascontiguousarray(np.asarray(h, dtype=np.float32))
    d = np.asarray(d).astype(np.int64)
    center = np.asarray(center, dtype=np.float32)

    order = np.argsort(d, kind="stable")
    ds = d[order]
    core_bounds = np.searchsorted(ds, np.arange(N_CORES + 1) * C_SHARD)
    counts_global = np.bincount(d, minlength=C)

    tile_rows_all = np.zeros((N_CORES, CT), dtype=np.int64)
    cnt_pads = []
    for c in range(N_CORES):
        cnt_pad = np.zeros(C_PAD, np.int64)
        cnt_pad[:C_SHARD] = counts_global[c * C_SHARD:(c + 1) * C_SHARD]
        cnt_pads.append(cnt_pad)
        tile_rows_all[c] = cnt_pad.reshape(CT, P).sum(1)

    n_j = np.maximum(np.ceil(tile_rows_all / P).astype(np.int64).max(0), 1)
    nbt = int(n_j.sum())
    h_groups = _h_groups(n_j)

    in_maps = []
    for c in range(N_CORES):
        lo, hi = core_bounds[c], core_bounds[c + 1]
        rows_idx = order[lo:hi]
        dl = (ds[lo:hi] - c * C_SHARD).astype(np.float32)
        tile_rows = tile_rows_all[c]
        tb = np.concatenate([[0], np.cumsum(tile_rows)])
        hs = h[rows_idx]  # this core's rows, class-sorted

        # per-class-tile pieces, partition-major [P, n_j, F] / [P, n_j]
        # d is pre-shifted by -128*j so the device compare is vs iota 0..127
        pieces_h, pieces_d = [], []
        for j in range(CT):
            r0, r1 = tb[j], tb[j + 1]
            nr = r1 - r0
            njj = int(n_j[j])
            bh = np.zeros((njj * P, F), np.float32)
            bd = np.zeros((njj * P,), np.float32)
            bh[:nr] = hs[r0:r1]
            bd[:nr] = dl[r0:r1] - j * P
            pieces_h.append(bh.reshape(njj, P, F).transpose(1, 0, 2))
            pieces_d.append(bd.reshape(njj, P).T)

        # h: concatenated per h-group, each group contiguous [P, cols*F]
        h_parts = []
        for g in h_groups:
            grp = np.concatenate([pieces_h[j] for j in g], axis=1)  # [P,cols,F]
            h_parts.append(grp.reshape(P, -1))
        h_blk = np.concatenate([p.reshape(-1) for p in h_parts])

        d_all = np.concatenate(pieces_d, axis=1)  # [P, nbt]

        counts_pm = cnt_pads[c].astype(np.float32).reshape(CT, P).T  # [P, CT]

        cshard = np.zeros((C_PAD, F), np.float32)
        cshard[:C_SHARD] = center[c * C_SHARD:(c + 1) * C_SHARD]
        # partition-major grouped center: [P, CT*F], tile t cols [t*F,(t+1)*F]
        center_pm = cshard.reshape(CT, P, F).transpose(1, 0, 2).reshape(P, CT * F)

        in_maps.append({
            "h_blk": np.ascontiguousarray(h_blk),
            "d_all": np.ascontiguousarray(d_all),
            "counts": np.ascontiguousarray(counts_pm),
            "center_in": np.ascontiguousarray(center_pm),
        })
    return in_maps, tuple(int(x) for x in n_j)


def _emit_body(nc, tc, mybir, pools, n_j, aps):
    """One full pass: segment sums + center update + loss partials."""
    dtf = mybir.dt.float32
    alu = mybir.AluOpType
    h_blk, d_all_d, counts, center_in, center_out, lossp = aps
    cpool, hpool, ohpool, clspool, scrpool, accpool, pspool, pslpool = pools

    nbt = int(sum(n_j))
    h_groups = _h_groups(n_j)
    c_groups = _c_groups()
    n_hg = len(h_groups)
    n_cg = len(c_groups)
    col_of = np.concatenate([[0], np.cumsum(n_j)]).astype(int)
    SCR_COLS = max(H_GROUP_COLS, C_GROUP) * F

    iota_i = cpool.tile([P, P], mybir.dt.int32, tag="iota_i")
    nc.gpsimd.iota(iota_i[:], pattern=[[1, P]], base=0, channel_multiplier=0)
    iota_f = cpool.tile([P, P], dtf, tag="iota_f")
    nc.vector.tensor_copy(iota_f[:], iota_i[:])
    ones = cpool.tile([P, 1], dtf, tag="ones")
    nc.vector.memset(ones[:], 1.0)

    # resident tables: pre-shifted class ids + per-class factors
    d_all = accpool.tile([P, nbt], dtf, tag="d_all")
    nc.sync.dma_start(d_all[:], d_all_d[:])
    cnt_all = accpool.tile([P, CT], dtf, tag="cnt_all")
    nc.sync.dma_start(cnt_all[:], counts[:])
    safe_all = accpool.tile([P, CT], dtf, tag="safe_all")
    nc.vector.tensor_scalar_max(safe_all[:], cnt_all[:], 1.0)
    inv_all = accpool.tile([P, CT], dtf, tag="inv_all")
    nc.vector.reciprocal(inv_all[:], safe_all[:])
    pres_all = accpool.tile([P, CT], dtf, tag="pres_all")  # 0/1 presence
    nc.vector.tensor_scalar(out=pres_all[:], in0=cnt_all[:], scalar1=0.5,
                            scalar2=None, op0=alu.is_ge)
    ainv_all = accpool.tile([P, CT], dtf, tag="ainv_all")  # alpha*present/cnt
    nc.vector.tensor_tensor(out=ainv_all[:], in0=pres_all[:], in1=inv_all[:],
                            op=alu.mult)
    nc.vector.tensor_scalar_mul(ainv_all[:], ainv_all[:], ALPHA)
    oma_all = accpool.tile([P, CT], dtf, tag="oma_all")    # 1 - alpha*present
    nc.vector.tensor_scalar(out=oma_all[:], in0=pres_all[:], scalar1=-ALPHA,
                            scalar2=1.0, op0=alu.mult, op1=alu.add)

    # loss staging columns
    acc = accpool.tile([P, 4], dtf, tag="acc")
    nc.vector.memset(acc[:], 0.0)
    hsq_g = accpool.tile([P, n_hg], dtf, tag="hsq_g")
    cross_g = accpool.tile([P, n_cg], dtf, tag="cross_g")
    csq_all = accpool.tile([P, CT], dtf, tag="csq_all")

    hgi = iter(enumerate(h_groups))
    ht = None
    ht_cols = ht_base = 0

    for cg, tiles in enumerate(c_groups):
        ncls = len(tiles)
        j0 = tiles[0]
        cin = clspool.tile([P, C_GROUP * F], dtf, tag="cin")
        nc.sync.dma_start(cin[:, :ncls * F],
                          center_in[:, j0 * F:(j0 + ncls) * F])
        cout = clspool.tile([P, C_GROUP * F], dtf, tag="cout")
        psg = pspool.tile([P, C_GROUP * F], dtf, tag="psg")

        for t, j in enumerate(tiles):
            nj = n_j[j]
            col = col_of[j]
            if ht is None or col >= ht_base + ht_cols:
                gidx, g = next(hgi)
                cols = int(sum(n_j[jj] for jj in g))
                ht = hpool.tile([P, H_GROUP_COLS * F], dtf, tag="ht")
                nc.sync.dma_start(
                    ht[:, :cols * F],
                    h_blk[P * col_of[g[0]] * F:
                          P * (col_of[g[0]] + cols) * F].rearrange(
                        "(p x) -> p x", p=P))
                ht_base, ht_cols = col, cols
                # sum_f h^2 over the whole group, one fused op
                scr = scrpool.tile([P, SCR_COLS], dtf, tag="scr")
                if gidx % ACT_HSQ_MOD == 0:
                    nc.scalar.activation(
                        scr[:, :cols * F], ht[:, :cols * F],
                        mybir.ActivationFunctionType.Square,
                        accum_out=hsq_g[:, gidx:gidx + 1])
                else:
                    nc.vector.scalar_tensor_tensor(
                        out=scr[:, :cols * F], in0=ht[:, :cols * F],
                        scalar=1.0, in1=ht[:, :cols * F],
                        op0=alu.mult, op1=alu.mult,
                        accum_out=hsq_g[:, gidx:gidx + 1])

            lc = col - ht_base
            # one-hot selector for all nj sub-tiles in one broadcast compare
            ohb = ohpool.tile([P, H_GROUP_COLS * P], dtf, tag="ohb")
            nc.vector.tensor_tensor(
                out=ohb[:, :nj * P],
                in0=d_all[:, col:col + nj].to_broadcast([P, nj, P]),
                in1=iota_f[:, None, :].broadcast_to([P, nj, P]),
                op=alu.is_equal)
            for s in range(nj):
                nc.tensor.matmul(
                    psg[:, t * F:(t + 1) * F],
                    lhsT=ohb[:, s * P:(s + 1) * P],
                    rhs=ht[:, (lc + s) * F:(lc + s + 1) * F],
                    start=(s == 0), stop=(s == nj - 1))

            # new_c = ainv*sums + oma*c  (t2 on Pool, fused stt on DVE)
            ctsl = cin[:, t * F:(t + 1) * F]
            t2 = clspool.tile([P, F], dtf, tag="t2")
            nc.gpsimd.tensor_scalar_mul(t2[:], ctsl, oma_all[:, j:j + 1])
            nc.vector.scalar_tensor_tensor(
                out=cout[:, t * F:(t + 1) * F], in0=psg[:, t * F:(t + 1) * F],
                scalar=ainv_all[:, j:j + 1], in1=t2[:],
                op0=alu.mult, op1=alu.add)
            # csq_all[:,j] = sum_f c^2 (ACT fused)
            csq = clspool.tile([P, F], dtf, tag="csq")
            nc.scalar.activation(csq[:], ctsl,
                                 mybir.ActivationFunctionType.Square,
                                 accum_out=csq_all[:, j:j + 1])

        # cross term for the whole group: sum_f,k sums*c, one fused op
        scr2 = scrpool.tile([P, SCR_COLS], dtf, tag="scr")
        nc.vector.scalar_tensor_tensor(
            out=scr2[:, :ncls * F], in0=psg[:, :ncls * F], scalar=1.0,
            in1=cin[:, :ncls * F], op0=alu.mult, op1=alu.mult,
            accum_out=cross_g[:, cg:cg + 1])

        nc.sync.dma_start(center_out[:, j0 * F:(j0 + ncls) * F],
                          cout[:, :ncls * F])

    # fold staging: acc[:,0]=sum h^2, acc[:,1]=sum s.c, acc[:,2]=sum n|c|^2
    nc.vector.tensor_reduce(acc[:, 0:1], hsq_g[:],
                            axis=mybir.AxisListType.X, op=alu.add)
    nc.vector.tensor_reduce(acc[:, 1:2], cross_g[:],
                            axis=mybir.AxisListType.X, op=alu.add)
    wc_all = accpool.tile([P, CT], dtf, tag="wc_all")
    nc.vector.tensor_tensor(out=wc_all[:], in0=csq_all[:], in1=cnt_all[:],
                            op=alu.mult)
    nc.vector.tensor_reduce(acc[:, 2:3], wc_all[:], axis=mybir.AxisListType.X,
                            op=alu.add)

    # reduce partition dim: lossp[1,4] = ones.T @ acc
    psl = pslpool.tile([1, 4], dtf, tag="psl")
    nc.tensor.matmul(psl[:], lhsT=ones[:], rhs=acc[:], start=True, stop=True)
    lt = accpool.tile([1, 4], dtf, tag="lt")
    nc.vector.tensor_copy(lt[:], psl[:])
    nc.sync.dma_start(lossp[:], lt[:])


def build_program(n_j, reps=1):
    """Compile the SPMD program for the given static per-class-tile batch
    sub-tile counts. reps>1 wraps the body in a hardware loop (timing)."""
    key = (tuple(n_j), reps)
    if key in _prog_cache:
        return _prog_cache[key]
    import concourse.bacc as bacc
    import concourse.mybir as mybir
    import concourse.tile as tile

    nbt = int(sum(n_j))
    dtf = mybir.dt.float32
    nc = bacc.Bacc("TRN2", target_bir_lowering=False, debug=False,
                   num_devices=N_CORES)
    h_blk = nc.dram_tensor("h_blk", [P * nbt * F], dtf,
                           kind="ExternalInput").ap()
    d_all = nc.dram_tensor("d_all", [P, nbt], dtf, kind="ExternalInput").ap()
    counts = nc.dram_tensor("counts", [P, CT], dtf, kind="ExternalInput").ap()
    center_in = nc.dram_tensor("center_in", [P, CT * F], dtf,
                               kind="ExternalInput").ap()
    center_out = nc.dram_tensor("center_out", [P, CT * F], dtf,
                                kind="ExternalOutput").ap()
    lossp = nc.dram_tensor("lossp", [1, 4], dtf, kind="ExternalOutput").ap()
    aps = (h_blk, d_all, counts, center_in, center_out, lossp)

    with tile.TileContext(nc) as tc:
        with (
            tc.tile_pool(name="const", bufs=1) as cpool,
            tc.tile_pool(name="hp", bufs=3) as hpool,
            tc.tile_pool(name="ohp", bufs=3) as ohpool,
            tc.tile_pool(name="cls", bufs=3) as clspool,
            tc.tile_pool(name="scrp", bufs=2) as scrpool,
            tc.tile_pool(name="accp", bufs=1) as accpool,
            tc.tile_pool(name="ps", bufs=2, space="PSUM") as pspool,
            tc.tile_pool(name="psl", bufs=1, space="PSUM") as pslpool,
        ):
            pools = (cpool, hpool, ohpool, clspool, scrpool, accpool,
                     pspool, pslpool)
            if reps == 1:
                _emit_body(nc, tc, mybir, pools, n_j, aps)
            else:
                with tc.For_i(0, reps, 1):
                    _emit_body(nc, tc, mybir, pools, n_j, aps)
    nc.compile()
    _prog_cache[key] = nc
    return nc


def _unshard(results):
    parts = []
    for c in range(N_CORES):
        pm = results[c]["center_out"].reshape(P, CT, F).transpose(1, 0, 2)
        parts.append(pm.reshape(C_PAD, F)[:C_SHARD])
    new_center = np.concatenate(parts, axis=0)
    tot = np.zeros(3, np.float64)
    for c in range(N_CORES):
        lp = results[c]["lossp"][0]
        tot += lp[:3].astype(np.float64)
    loss = (tot[0] - 2.0 * tot[1] + tot[2]) / (BATCH * F)
    return np.float32(loss), new_center


def kernel(h, d, center):
    from concourse.bass_utils import run_bass_kernel_spmd

    in_maps, n_j = _shard_inputs(h, d, center)
    nc = build_program(n_j, reps=1)
    res = run_bass_kernel_spmd(nc, in_maps, core_ids=list(range(N_CORES)))
    return _unshard(res.results)


# revision 9
# speedup vs baseline: 2.6215x; 2.6215x over previous
"""CenterLoss kernel for 8 Trainium2 NeuronCores (Bass/Tile).

Problem (hardcoded, from nn_CenterLoss):
    h      [262144, 256] f32
    d      [262144]      int   (class ids in [0, 100000))
    center [100000, 256] f32
    returns (loss, new_center):
        loss       = mean((h - center[d])**2)                       scalar f32
        new_center = center + 0.1 * where(count>0, segmean(h)-center, 0)

Sharding strategy: shard along n_class. Core c owns classes
[c*12500, (c+1)*12500) and the matching center rows; the host routes each
batch row to the core owning its class (sorted by class id). Segment sums
then never need a cross-device reduction.

Device algorithm per core:
  - Classes are processed in 98 tiles of 128. For class tile j the batch
    rows belonging to it (padded with zero rows to n_j*128, n_j static and
    identical on every core so the SPMD program is uniform) are multiplied
    by an on-device-built one-hot selector and accumulated into PSUM:
        sums[128, 256] += onehot[128b, 128c].T @ h[128b, 256]
    The selector for all n_j sub-tiles is built with ONE broadcast
    is_equal compare against a resident iota tile (host pre-shifts the
    class ids by -128j so the compare needs no per-tile offset).
  - loss uses the expansion sum|h|^2 - 2*sum_k s_k.c_k + sum_k n_k|c_k|^2
    so center[d] is never gathered. The dot products are fused
    multiply+free-dim-reduce ops (scalar_tensor_tensor accum_out / ACT
    Square accum_out) over wide group tiles.
  - new_center tile = ainv*sums + oma*c in one fused stt op per tile,
    with ainv = alpha*present/count, oma = 1 - alpha*present per class.
  - All DRAM streams are laid out partition-major on the host so each
    multi-tile group moves with a single fully-contiguous DMA.
"""

import numpy as np

BATCH = 262144
F = 256
C = 100000
ALPHA = 0.1
N_CORES = 8
P = 128
C_SHARD = C // N_CORES          # 12500 classes per core
CT = (C_SHARD + P - 1) // P     # 98 class tiles per core
C_PAD = CT * P                  # 12544 rows incl. padding

H_GROUP_COLS = 12               # max batch sub-tiles per h DMA group
C_GROUP = 4                     # class tiles per center/PSUM group
ACT_HSQ_MOD = 2                 # h-group idx % MOD == 0 -> |h|^2 on ACT

_prog_cache = {}


def _h_groups(n_j):
    """Greedily group consecutive class tiles so each h DMA moves at most
    H_GROUP_COLS 128-row sub-tiles."""
    groups = []
    cur = []
    cols = 0
    for j, nj in enumerate(n_j):
        if cur and cols + nj > H_GROUP_COLS:
            groups.append(cur)
            cur, cols = [], 0
        cur.append(j)
        cols += nj
    if cur:
        groups.append(cur)
    return groups


def _c_groups():
    return [list(range(g, min(g + C_GROUP, CT))) for g in range(0, CT, C_GROUP)]


def _shard_inputs(h, d, center):
    """Route batch rows to the core owning their class; build per-core
    partition-major blocks so every device DMA is contiguous.

    Returns (in_maps, n_j)."""
    h = np.ascontiguousarray(np.asarray(h, dtype=np.float32))
    d = np.asarray(d).astype(np.int64)
    center = np.asarray(center, dtype=np.float32)

    order = np.argsort(d, kind="stable")
    ds = d[order]
    core_bounds = np.searchsorted(ds, np.arange(N_CORES + 1) * C_SHARD)
    counts_global = np.bincount(d, minlength=C)

    tile_rows_all = np.zeros((N_CORES, CT), dtype=np.int64)
    cnt_pads = []
    for c in range(N_CORES):
        cnt_pad = np.zeros(C_PAD, np.int64)
        cnt_pad[:C_SHARD] = counts_global[c * C_SHARD:(c + 1) * C_SHARD]
        cnt_pads.append(cnt_pad)
        tile_rows_all[c] = cnt_pad.reshape(CT, P).sum(1)

    n_j = np.maximum(np.ceil(tile_rows_all / P).astype(np.int64).max(0), 1)
    nbt = int(n_j.sum())
    h_groups = _h_groups(n_j)

    in_maps = []
    for c in range(N_CORES):
        lo, hi = core_bounds[c], core_bounds[c + 1]
        rows_idx = order[lo:hi]
        dl = (ds[lo:hi] - c * C_SHARD).astype(np.float32)
        tile_rows = tile_rows_all[c]
        tb = np.concatenate([[0], np.cumsum(tile_rows)])
        hs = h[rows_idx]  # this core's rows, class-sorted

        # per-class-tile pieces, partition-major [P, n_j, F] / [P, n_j]
        # d is pre-shifted by -128*j so the device compare is vs iota 0..127
        pieces_h, pieces_d = [], []
        for j in range(CT):
            r0, r1 = tb[j], tb[j + 1]
            nr = r1 - r0
            njj = int(n_j[j])
            bh = np.zeros((njj * P, F), np.float32)
            bd = np.zeros((njj * P,), np.float32)
            bh[:nr] = hs[r0:r1]
            bd[:nr] = dl[r0:r1] - j * P
            pieces_h.append(bh.reshape(njj, P, F).transpose(1, 0, 2))
            pieces_d.append(bd.reshape(njj, P).T)

        # h: concatenated per h-group, each group contiguous [P, cols*F]
        h_parts = []
        for g in h_groups:
            grp = np.concatenate([pieces_h[j] for j in g], axis=1)  # [P,cols,F]
            h_parts.append(grp.reshape(P, -1))
        h_blk = np.concatenate([p.reshape(-1) for p in h_parts])

        d_all = np.concatenate(pieces_d, axis=1)  # [P, nbt]

        counts_pm = cnt_pads[c].astype(np.float32).reshape(CT, P).T  # [P, CT]

        cshard = np.zeros((C_PAD, F), np.float32)
        cshard[:C_SHARD] = center[c * C_SHARD:(c + 1) * C_SHARD]
        # partition-major grouped center: [P, CT*F], tile t cols [t*F,(t+1)*F]
        center_pm = cshard.reshape(CT, P, F).transpose(1, 0, 2).reshape(P, CT * F)

        in_maps.append({
            "h_blk": np.ascontiguousarray(h_blk),
            "d_all": np.ascontiguousarray(d_all),
            "counts": np.ascontiguousarray(counts_pm),
            "center_in": np.ascontiguousarray(center_pm),
        })
    return in_maps, tuple(int(x) for x in n_j)


def _emit_body(nc, tc, mybir, pools, n_j, aps):
    """One full pass: segment sums + center update + loss partials."""
    dtf = mybir.dt.float32
    alu = mybir.AluOpType
    h_blk, d_all_d, counts, center_in, center_out, lossp = aps
    cpool, hpool, ohpool, clspool, scrpool, accpool, pspool, pslpool = pools

    nbt = int(sum(n_j))
    h_groups = _h_groups(n_j)
    c_groups = _c_groups()
    n_hg = len(h_groups)
    n_cg = len(c_groups)
    col_of = np.concatenate([[0], np.cumsum(n_j)]).astype(int)
    SCR_COLS = max(H_GROUP_COLS, C_GROUP) * F
    MAX_NJ = int(max(n_j))

    iota_i = cpool.tile([P, P], mybir.dt.int32, tag="iota_i")
    nc.gpsimd.iota(iota_i[:], pattern=[[1, P]], base=0, channel_multiplier=0)
    iota_f = cpool.tile([P, P], dtf, tag="iota_f")
    nc.vector.tensor_copy(iota_f[:], iota_i[:])
    ones = cpool.tile([P, 1], dtf, tag="ones")
    nc.vector.memset(ones[:], 1.0)

    # resident tables: pre-shifted class ids + per-class factors
    d_all = accpool.tile([P, nbt], dtf, tag="d_all")
    nc.sync.dma_start(d_all[:], d_all_d[:])
    cnt_all = accpool.tile([P, CT], dtf, tag="cnt_all")
    nc.sync.dma_start(cnt_all[:], counts[:])
    safe_all = accpool.tile([P, CT], dtf, tag="safe_all")
    nc.vector.tensor_scalar_max(safe_all[:], cnt_all[:], 1.0)
    inv_all = accpool.tile([P, CT], dtf, tag="inv_all")
    nc.vector.reciprocal(inv_all[:], safe_all[:])
    pres_all = accpool.tile([P, CT], dtf, tag="pres_all")  # 0/1 presence
    nc.vector.tensor_scalar(out=pres_all[:], in0=cnt_all[:], scalar1=0.5,
                            scalar2=None, op0=alu.is_ge)
    ainv_all = accpool.tile([P, CT], dtf, tag="ainv_all")  # alpha*present/cnt
    nc.vector.tensor_tensor(out=ainv_all[:], in0=pres_all[:], in1=inv_all[:],
                            op=alu.mult)
    nc.vector.tensor_scalar_mul(ainv_all[:], ainv_all[:], ALPHA)
    oma_all = accpool.tile([P, CT], dtf, tag="oma_all")    # 1 - alpha*present
    nc.vector.tensor_scalar(out=oma_all[:], in0=pres_all[:], scalar1=-ALPHA,
                            scalar2=1.0, op0=alu.mult, op1=alu.add)

    # loss staging columns
    acc = accpool.tile([P, 4], dtf, tag="acc")
    nc.vector.memset(acc[:], 0.0)
    hsq_g = accpool.tile([P, n_hg], dtf, tag="hsq_g")
    cross_g = accpool.tile([P, n_cg], dtf, tag="cross_g")
    csq_all = accpool.tile([P, CT], dtf, tag="csq_all")

    hgi = iter(enumerate(h_groups))
    ht = None
    ht_cols = ht_base = 0

    for cg, tiles in enumerate(c_groups):
        ncls = len(tiles)
        j0 = tiles[0]
        cin = clspool.tile([P, C_GROUP * F], dtf, tag="cin")
        nc.sync.dma_start(cin[:, :ncls * F],
                          center_in[:, j0 * F:(j0 + ncls) * F])
        cout = clspool.tile([P, C_GROUP * F], dtf, tag="cout")
        psg = pspool.tile([P, C_GROUP * F], dtf, tag="psg")

        for t, j in enumerate(tiles):
            nj = n_j[j]
            col = col_of[j]
            if ht is None or col >= ht_base + ht_cols:
                gidx, g = next(hgi)
                cols = int(sum(n_j[jj] for jj in g))
                ht = hpool.tile([P, H_GROUP_COLS * F], dtf, tag="ht")
                nc.sync.dma_start(
                    ht[:, :cols * F],
                    h_blk[P * col_of[g[0]] * F:
                          P * (col_of[g[0]] + cols) * F].rearrange(
                        "(p x) -> p x", p=P))
                ht_base, ht_cols = col, cols
                # sum_f h^2 over the whole group, one fused op
                scr = scrpool.tile([P, SCR_COLS], dtf, tag="scr")
                if gidx % ACT_HSQ_MOD == 0:
                    nc.scalar.activation(
                        scr[:, :cols * F], ht[:, :cols * F],
                        mybir.ActivationFunctionType.Square,
                        accum_out=hsq_g[:, gidx:gidx + 1])
                else:
                    nc.vector.scalar_tensor_tensor(
                        out=scr[:, :cols * F], in0=ht[:, :cols * F],
                        scalar=1.0, in1=ht[:, :cols * F],
                        op0=alu.mult, op1=alu.mult,
                        accum_out=hsq_g[:, gidx:gidx + 1])

            lc = col - ht_base
            # one-hot selector for all nj sub-tiles in one broadcast compare
            ohb = ohpool.tile([P, MAX_NJ * P], dtf, tag="ohb")
            nc.vector.tensor_tensor(
                out=ohb[:, :nj * P],
                in0=d_all[:, col:col + nj].to_broadcast([P, nj, P]),
                in1=iota_f[:, None, :].broadcast_to([P, nj, P]),
                op=alu.is_equal)
            for s in range(nj):
                nc.tensor.matmul(
                    psg[:, t * F:(t + 1) * F],
                    lhsT=ohb[:, s * P:(s + 1) * P],
                    rhs=ht[:, (lc + s) * F:(lc + s + 1) * F],
                    start=(s == 0), stop=(s == nj - 1))

            # new_c = ainv*sums + oma*c  (t2 on Pool, fused stt on DVE)
            ctsl = cin[:, t * F:(t + 1) * F]
            t2 = clspool.tile([P, F], dtf, tag="t2")
            nc.gpsimd.tensor_scalar_mul(t2[:], ctsl, oma_all[:, j:j + 1])
            nc.vector.scalar_tensor_tensor(
                out=cout[:, t * F:(t + 1) * F], in0=psg[:, t * F:(t + 1) * F],
                scalar=ainv_all[:, j:j + 1], in1=t2[:],
                op0=alu.mult, op1=alu.add)
            # csq_all[:,j] = sum_f c^2 (ACT fused)
            csq = clspool.tile([P, F], dtf, tag="csq")
            nc.scalar.activation(csq[:], ctsl,
                                 mybir.ActivationFunctionType.Square,
                                 accum_out=csq_all[:, j:j + 1])

        # cross term for the whole group: sum_f,k sums*c, one fused op
        scr2 = scrpool.tile([P, SCR_COLS], dtf, tag="scr")
        nc.vector.scalar_tensor_tensor(
            out=scr2[:, :ncls * F], in0=psg[:, :ncls * F], scalar=1.0,
            in1=cin[:, :ncls * F], op0=alu.mult, op1=alu.mult,
            accum_out=cross_g[:, cg:cg + 1])

        nc.sync.dma_start(center_out[:, j0 * F:(j0 + ncls) * F],
                          cout[:, :ncls * F])

    # fold staging: acc[:,0]=sum h^2, acc[:,1]=sum s.c, acc[:,2]=sum n|c|^2
    nc.vector.tensor_reduce(acc[:, 0:1], hsq_g[:],
                            axis=mybir.AxisListType.X, op=alu.add)
    nc.vector.tensor_reduce(acc[:, 1:2], cross_g[:],
                            axis=mybir.AxisListType.X, op=alu.add)
    wc_all = accpool.tile([P, CT], dtf, tag="wc_all")
    nc.vector.tensor_tensor(out=wc_all[:], in0=csq_all[:], in1=cnt_all[:],
                            op=alu.mult)
    nc.vector.tensor_reduce(acc[:, 2:3], wc_all[:], axis=mybir.AxisListType.X,
                            op=alu.add)

    # reduce partition dim: lossp[1,4] = ones.T @ acc
    psl = pslpool.tile([1, 4], dtf, tag="psl")
    nc.tensor.matmul(psl[:], lhsT=ones[:], rhs=acc[:], start=True, stop=True)
    lt = accpool.tile([1, 4], dtf, tag="lt")
    nc.vector.tensor_copy(lt[:], psl[:])
    nc.sync.dma_start(lossp[:], lt[:])


def build_program(n_j, reps=1):
    """Compile the SPMD program for the given static per-class-tile batch
    sub-tile counts. reps>1 wraps the body in a hardware loop (timing)."""
    key = (tuple(n_j), reps)
    if key in _prog_cache:
        return _prog_cache[key]
    import concourse.bacc as bacc
    import concourse.mybir as mybir
    import concourse.tile as tile

    nbt = int(sum(n_j))
    dtf = mybir.dt.float32
    nc = bacc.Bacc("TRN2", target_bir_lowering=False, debug=False,
                   num_devices=N_CORES)
    h_blk = nc.dram_tensor("h_blk", [P * nbt * F], dtf,
                           kind="ExternalInput").ap()
    d_all = nc.dram_tensor("d_all", [P, nbt], dtf, kind="ExternalInput").ap()
    counts = nc.dram_tensor("counts", [P, CT], dtf, kind="ExternalInput").ap()
    center_in = nc.dram_tensor("center_in", [P, CT * F], dtf,
                               kind="ExternalInput").ap()
    center_out = nc.dram_tensor("center_out", [P, CT * F], dtf,
                                kind="ExternalOutput").ap()
    lossp = nc.dram_tensor("lossp", [1, 4], dtf, kind="ExternalOutput").ap()
    aps = (h_blk, d_all, counts, center_in, center_out, lossp)

    with tile.TileContext(nc) as tc:
        with (
            tc.tile_pool(name="const", bufs=1) as cpool,
            tc.tile_pool(name="hp", bufs=4) as hpool,
            tc.tile_pool(name="ohp", bufs=4) as ohpool,
            tc.tile_pool(name="cls", bufs=4) as clspool,
            tc.tile_pool(name="scrp", bufs=3) as scrpool,
            tc.tile_pool(name="accp", bufs=1) as accpool,
            tc.tile_pool(name="ps", bufs=3, space="PSUM") as pspool,
            tc.tile_pool(name="psl", bufs=1, space="PSUM") as pslpool,
        ):
            pools = (cpool, hpool, ohpool, clspool, scrpool, accpool,
                     pspool, pslpool)
            if reps == 1:
                _emit_body(nc, tc, mybir, pools, n_j, aps)
            else:
                with tc.For_i(0, reps, 1):
                    _emit_body(nc, tc, mybir, pools, n_j, aps)
    nc.compile()
    _prog_cache[key] = nc
    return nc


def _unshard(results):
    parts = []
    for c in range(N_CORES):
        pm = results[c]["center_out"].reshape(P, CT, F).transpose(1, 0, 2)
        parts.append(pm.reshape(C_PAD, F)[:C_SHARD])
    new_center = np.concatenate(parts, axis=0)
    tot = np.zeros(3, np.float64)
    for c in range(N_CORES):
        lp = results[c]["lossp"][0]
        tot += lp[:3].astype(np.float64)
    loss = (tot[0] - 2.0 * tot[1] + tot[2]) / (BATCH * F)
    return np.float32(loss), new_center


def kernel(h, d, center):
    from concourse.bass_utils import run_bass_kernel_spmd

    in_maps, n_j = _shard_inputs(h, d, center)
    nc = build_program(n_j, reps=1)
    res = run_bass_kernel_spmd(nc, in_maps, core_ids=list(range(N_CORES)))
    return _unshard(res.results)


# revision 16
# speedup vs baseline: 4.7481x; 1.8112x over previous
"""CenterLoss kernel for 8 Trainium2 NeuronCores (Bass/Tile).

Problem (hardcoded, from nn_CenterLoss):
    h      [262144, 256] f32
    d      [262144]      int   (class ids in [0, 100000))
    center [100000, 256] f32
    returns (loss, new_center):
        loss       = mean((h - center[d])**2)                       scalar f32
        new_center = center + 0.1 * where(count>0, segmean(h)-center, 0)

Sharding strategy: shard along n_class. Core c owns classes
[c*12500, (c+1)*12500) and the matching center rows; the host routes each
batch row to the core owning its class (sorted by class id). Segment sums
then never need a cross-device reduction.

Device algorithm per core:
  - Classes are processed in 98 tiles of 128. For class tile j the batch
    rows belonging to it (padded with zero rows to n_j*128, n_j static and
    identical on every core so the SPMD program is uniform) are multiplied
    by an on-device-built one-hot selector and accumulated into PSUM:
        sums[128, 256] += onehot[128b, 128c].T @ h[128b, 256]
    The selector for all n_j sub-tiles is built with ONE broadcast
    is_equal compare against a resident iota tile (host pre-shifts the
    class ids by -128j so the compare needs no per-tile offset).
  - loss uses the expansion sum|h|^2 - 2*sum_k s_k.c_k + sum_k n_k|c_k|^2
    so center[d] is never gathered. The dot products are fused
    multiply+free-dim-reduce ops (scalar_tensor_tensor accum_out / ACT
    Square accum_out) over wide group tiles.
  - new_center tile = ainv*sums + oma*c: oma*c on the scalar engine
    (per-partition scale), then one fused stt op on the vector engine.
    ainv = alpha*present/count, oma = 1 - alpha*present per class.
  - All DRAM streams are laid out partition-major on the host so each
    multi-tile group moves with a single fully-contiguous DMA.
"""

import numpy as np

BATCH = 262144
F = 256
C = 100000
ALPHA = 0.1
N_CORES = 8
P = 128
C_SHARD = C // N_CORES          # 12500 classes per core
CT = (C_SHARD + P - 1) // P     # 98 class tiles per core
C_PAD = CT * P                  # 12544 rows incl. padding

H_GROUP_COLS = 12               # max batch sub-tiles per h DMA group
C_GROUP = 3                     # class tiles per PSUM group
CIN_PSGROUPS = 3                # PSUM groups per center DMA group
PSB = 512                       # PSUM bank stride (f32) per class tile
ACT_HSQ_MOD = 2                 # h-group idx % MOD == 0 -> |h|^2 on ACT
T2_ENGINE = "act"               # 'pool' | 'act': engine for oma*c

_prog_cache = {}


def _h_groups(n_j):
    """Greedily group consecutive class tiles so each h DMA moves at most
    H_GROUP_COLS 128-row sub-tiles."""
    groups = []
    cur = []
    cols = 0
    for j, nj in enumerate(n_j):
        if cur and cols + nj > H_GROUP_COLS:
            groups.append(cur)
            cur, cols = [], 0
        cur.append(j)
        cols += nj
    if cur:
        groups.append(cur)
    return groups


def _c_groups():
    return [list(range(g, min(g + C_GROUP, CT))) for g in range(0, CT, C_GROUP)]


def _shard_inputs(h, d, center):
    """Route batch rows to the core owning their class; build per-core
    partition-major blocks so every device DMA is contiguous.

    Returns (in_maps, n_j)."""
    h = np.ascontiguousarray(np.asarray(h, dtype=np.float32))
    d = np.asarray(d).astype(np.int64)
    center = np.asarray(center, dtype=np.float32)

    order = np.argsort(d, kind="stable")
    ds = d[order]
    core_bounds = np.searchsorted(ds, np.arange(N_CORES + 1) * C_SHARD)
    counts_global = np.bincount(d, minlength=C)

    tile_rows_all = np.zeros((N_CORES, CT), dtype=np.int64)
    cnt_pads = []
    for c in range(N_CORES):
        cnt_pad = np.zeros(C_PAD, np.int64)
        cnt_pad[:C_SHARD] = counts_global[c * C_SHARD:(c + 1) * C_SHARD]
        cnt_pads.append(cnt_pad)
        tile_rows_all[c] = cnt_pad.reshape(CT, P).sum(1)

    n_j = np.maximum(np.ceil(tile_rows_all / P).astype(np.int64).max(0), 1)
    nbt = int(n_j.sum())
    h_groups = _h_groups(n_j)

    in_maps = []
    for c in range(N_CORES):
        lo, hi = core_bounds[c], core_bounds[c + 1]
        rows_idx = order[lo:hi]
        dl = (ds[lo:hi] - c * C_SHARD).astype(np.float32)
        tile_rows = tile_rows_all[c]
        tb = np.concatenate([[0], np.cumsum(tile_rows)])
        hs = h[rows_idx]  # this core's rows, class-sorted

        # per-class-tile pieces, partition-major [P, n_j, F] / [P, n_j]
        # d is pre-shifted by -128*j so the device compare is vs iota 0..127
        pieces_h, pieces_d = [], []
        for j in range(CT):
            r0, r1 = tb[j], tb[j + 1]
            nr = r1 - r0
            njj = int(n_j[j])
            bh = np.zeros((njj * P, F), np.float32)
            bd = np.zeros((njj * P,), np.float32)
            bh[:nr] = hs[r0:r1]
            bd[:nr] = dl[r0:r1] - j * P
            pieces_h.append(bh.reshape(njj, P, F).transpose(1, 0, 2))
            pieces_d.append(bd.reshape(njj, P).T)

        # h: concatenated per h-group, each group contiguous [P, cols*F]
        h_parts = []
        for g in h_groups:
            grp = np.concatenate([pieces_h[j] for j in g], axis=1)  # [P,cols,F]
            h_parts.append(grp.reshape(P, -1))
        h_blk = np.concatenate([p.reshape(-1) for p in h_parts])

        d_all = np.concatenate(pieces_d, axis=1)  # [P, nbt]

        counts_pm = cnt_pads[c].astype(np.float32).reshape(CT, P).T  # [P, CT]

        cshard = np.zeros((C_PAD, F), np.float32)
        cshard[:C_SHARD] = center[c * C_SHARD:(c + 1) * C_SHARD]
        # partition-major grouped center: [P, CT*F], tile t cols [t*F,(t+1)*F]
        center_pm = cshard.reshape(CT, P, F).transpose(1, 0, 2).reshape(P, CT * F)

        in_maps.append({
            "h_blk": np.ascontiguousarray(h_blk),
            "d_all": np.ascontiguousarray(d_all),
            "counts": np.ascontiguousarray(counts_pm),
            "center_in": np.ascontiguousarray(center_pm),
        })
    return in_maps, tuple(int(x) for x in n_j)


def _emit_body(nc, tc, mybir, pools, n_j, aps, strip=0):
    """One full pass: segment sums + center update + loss partials."""
    dtf = mybir.dt.float32
    alu = mybir.AluOpType
    h_blk, d_all_d, counts, center_in, center_out, lossp = aps
    cpool, hpool, ohpool, clspool, scrpool, accpool, pspool, pslpool = pools

    nbt = int(sum(n_j))
    h_groups = _h_groups(n_j)
    c_groups = _c_groups()
    n_hg = len(h_groups)
    n_cg = len(c_groups)
    col_of = np.concatenate([[0], np.cumsum(n_j)]).astype(int)
    SCR_COLS = max(H_GROUP_COLS, C_GROUP) * F
    MAX_NJ = int(max(n_j))

    iota_i = cpool.tile([P, P], mybir.dt.int32, tag="iota_i")
    nc.gpsimd.iota(iota_i[:], pattern=[[1, P]], base=0, channel_multiplier=0)
    iota_f = cpool.tile([P, P], dtf, tag="iota_f")
    nc.vector.tensor_copy(iota_f[:], iota_i[:])
    ones = cpool.tile([P, 1], dtf, tag="ones")
    nc.vector.memset(ones[:], 1.0)

    # resident tables: pre-shifted class ids + per-class factors
    d_all = accpool.tile([P, nbt], dtf, tag="d_all")
    nc.sync.dma_start(d_all[:], d_all_d[:])
    cnt_all = accpool.tile([P, CT], dtf, tag="cnt_all")
    nc.sync.dma_start(cnt_all[:], counts[:])
    safe_all = accpool.tile([P, CT], dtf, tag="safe_all")
    nc.vector.tensor_scalar_max(safe_all[:], cnt_all[:], 1.0)
    inv_all = accpool.tile([P, CT], dtf, tag="inv_all")
    nc.vector.reciprocal(inv_all[:], safe_all[:])
    pres_all = accpool.tile([P, CT], dtf, tag="pres_all")  # 0/1 presence
    nc.vector.tensor_scalar(out=pres_all[:], in0=cnt_all[:], scalar1=0.5,
                            scalar2=None, op0=alu.is_ge)
    ainv_all = accpool.tile([P, CT], dtf, tag="ainv_all")  # alpha*present/cnt
    nc.vector.tensor_tensor(out=ainv_all[:], in0=pres_all[:], in1=inv_all[:],
                            op=alu.mult)
    nc.vector.tensor_scalar_mul(ainv_all[:], ainv_all[:], ALPHA)
    oma_all = accpool.tile([P, CT], dtf, tag="oma_all")    # 1 - alpha*present
    nc.vector.tensor_scalar(out=oma_all[:], in0=pres_all[:], scalar1=-ALPHA,
                            scalar2=1.0, op0=alu.mult, op1=alu.add)

    # loss staging columns
    acc = accpool.tile([P, 4], dtf, tag="acc")
    nc.vector.memset(acc[:], 0.0)
    hsq_g = accpool.tile([P, n_hg], dtf, tag="hsq_g")
    cross_g = accpool.tile([P, n_cg], dtf, tag="cross_g")
    csq_all = accpool.tile([P, CT], dtf, tag="csq_all")

    hgi = iter(enumerate(h_groups))
    ht = None
    ht_cols = ht_base = 0
    ohb = None

    cin_groups = [c_groups[i:i + CIN_PSGROUPS]
                  for i in range(0, len(c_groups), CIN_PSGROUPS)]
    cg = 0
    for cgrp in cin_groups:
        jc0 = cgrp[0][0]
        ntot = sum(len(t) for t in cgrp)
        cin = clspool.tile([P, CIN_PSGROUPS * C_GROUP * F], dtf, tag="cin")
        nc.sync.dma_start(cin[:, :ntot * F],
                          center_in[:, jc0 * F:(jc0 + ntot) * F])
        cout = clspool.tile([P, CIN_PSGROUPS * C_GROUP * F], dtf, tag="cout")

        for tiles in cgrp:
            ncls = len(tiles)
            j0 = tiles[0]
            psg = pspool.tile([P, C_GROUP * PSB], dtf, tag="psg")

            for t, j in enumerate(tiles):
                nj = n_j[j]
                col = col_of[j]
                if ht is None or col >= ht_base + ht_cols:
                    gidx, g = next(hgi)
                    cols = int(sum(n_j[jj] for jj in g))
                    ht = hpool.tile([P, H_GROUP_COLS * F], dtf, tag="ht")
                    nc.sync.dma_start(
                        ht[:, :cols * F],
                        h_blk[P * col_of[g[0]] * F:
                              P * (col_of[g[0]] + cols) * F].rearrange(
                            "(p x) -> p x", p=P))
                    ht_base, ht_cols = col, cols
                    if strip < 3 or strip == 5:
                        # one-hot selectors for ALL sub-tiles in the
                        # h-group, one broadcast compare
                        ohb = ohpool.tile([P, H_GROUP_COLS * P], dtf,
                                          tag="ohb")
                        nc.vector.tensor_tensor(
                            out=ohb[:, :cols * P],
                            in0=d_all[:, col:col + cols].to_broadcast(
                                [P, cols, P]),
                            in1=iota_f[:, None, :].broadcast_to(
                                [P, cols, P]),
                            op=alu.is_equal)
                    # sum_f h^2 over the whole group, one fused op
                    scr = scrpool.tile([P, SCR_COLS], dtf, tag="scr")
                    if strip >= 1:
                        pass
                    elif gidx % ACT_HSQ_MOD == 0:
                        nc.scalar.activation(
                            scr[:, :cols * F], ht[:, :cols * F],
                            mybir.ActivationFunctionType.Square,
                            accum_out=hsq_g[:, gidx:gidx + 1])
                    else:
                        nc.vector.scalar_tensor_tensor(
                            out=scr[:, :cols * F], in0=ht[:, :cols * F],
                            scalar=1.0, in1=ht[:, :cols * F],
                            op0=alu.mult, op1=alu.mult,
                            accum_out=hsq_g[:, gidx:gidx + 1])

                lc = col - ht_base
                if strip < 4 or strip == 5:
                    for s in range(nj):
                        nc.tensor.matmul(
                            psg[:, t * PSB:t * PSB + F],
                            lhsT=(ohb[:, (lc + s) * P:(lc + s + 1) * P]
                                  if (strip < 3 or strip == 5)
                                  else iota_f[:]),
                            rhs=ht[:, (lc + s) * F:(lc + s + 1) * F],
                            start=(s == 0), stop=(s == nj - 1))

                # new_c = ainv*sums + oma*c (oma*c on ACT, fused stt on DVE)
                tc_ = j - jc0
                ctsl = cin[:, tc_ * F:(tc_ + 1) * F]
                if strip < 2 or strip == 5:
                    if strip == 5:
                        t2 = ctsl  # timing probe: skip the oma pre-scale
                    else:
                        t2 = clspool.tile([P, F], dtf, tag="t2")
                        if T2_ENGINE == "pool":
                            nc.gpsimd.tensor_scalar_mul(
                                t2[:], ctsl, oma_all[:, j:j + 1])
                        else:
                            nc.scalar.activation(
                                t2[:], ctsl,
                                mybir.ActivationFunctionType.Copy,
                                scale=oma_all[:, j:j + 1])
                        t2 = t2[:]
                    nc.vector.scalar_tensor_tensor(
                        out=cout[:, tc_ * F:(tc_ + 1) * F],
                        in0=psg[:, t * PSB:t * PSB + F],
                        scalar=ainv_all[:, j:j + 1], in1=t2,
                        op0=alu.mult, op1=alu.add)
                if strip < 1:
                    # csq_all[:,j] = sum_f c^2 (ACT fused)
                    csq = clspool.tile([P, F], dtf, tag="csq")
                    nc.scalar.activation(
                        csq[:], ctsl, mybir.ActivationFunctionType.Square,
                        accum_out=csq_all[:, j:j + 1])

            if strip < 1:
                # cross term for the whole PSUM group, one fused op
                tg0 = (j0 - jc0) * F
                scr2 = scrpool.tile([P, SCR_COLS], dtf, tag="scr")
                psg_v = psg[:].rearrange("p (a b) -> p a b", b=PSB)[:, :ncls, :F]
                nc.vector.scalar_tensor_tensor(
                    out=scr2[:, :ncls * F].rearrange("p (a f) -> p a f", f=F),
                    in0=psg_v, scalar=1.0,
                    in1=cin[:, tg0:tg0 + ncls * F].rearrange(
                        "p (a f) -> p a f", f=F),
                    op0=alu.mult, op1=alu.mult,
                    accum_out=cross_g[:, cg:cg + 1])
            cg += 1

        nc.sync.dma_start(
            center_out[:, jc0 * F:(jc0 + ntot) * F],
            (cout if (strip < 2 or strip == 5) else cin)[:, :ntot * F])

    # fold staging: acc[:,0]=sum h^2, acc[:,1]=sum s.c, acc[:,2]=sum n|c|^2
    if strip >= 1:
        lt = accpool.tile([1, 4], dtf, tag="lt")
        nc.vector.memset(lt[:], 0.0)
        nc.sync.dma_start(lossp[:], lt[:])
        return
    nc.vector.tensor_reduce(acc[:, 0:1], hsq_g[:],
                            axis=mybir.AxisListType.X, op=alu.add)
    nc.vector.tensor_reduce(acc[:, 1:2], cross_g[:],
                            axis=mybir.AxisListType.X, op=alu.add)
    wc_all = accpool.tile([P, CT], dtf, tag="wc_all")
    nc.vector.tensor_tensor(out=wc_all[:], in0=csq_all[:], in1=cnt_all[:],
                            op=alu.mult)
    nc.vector.tensor_reduce(acc[:, 2:3], wc_all[:], axis=mybir.AxisListType.X,
                            op=alu.add)

    # reduce partition dim: lossp[1,4] = ones.T @ acc
    psl = pslpool.tile([1, 4], dtf, tag="psl")
    nc.tensor.matmul(psl[:], lhsT=ones[:], rhs=acc[:], start=True, stop=True)
    lt = accpool.tile([1, 4], dtf, tag="lt")
    nc.vector.tensor_copy(lt[:], psl[:])
    nc.sync.dma_start(lossp[:], lt[:])


def build_program(n_j, reps=1, strip=0):
    """Compile the SPMD program for the given static per-class-tile batch
    sub-tile counts. reps>1 wraps the body in a hardware loop (timing)."""
    key = (tuple(n_j), reps, strip)
    if key in _prog_cache:
        return _prog_cache[key]
    import concourse.bacc as bacc
    import concourse.mybir as mybir
    import concourse.tile as tile

    nbt = int(sum(n_j))
    dtf = mybir.dt.float32
    nc = bacc.Bacc("TRN2", target_bir_lowering=False, debug=False,
                   num_devices=N_CORES)
    h_blk = nc.dram_tensor("h_blk", [P * nbt * F], dtf,
                           kind="ExternalInput").ap()
    d_all = nc.dram_tensor("d_all", [P, nbt], dtf, kind="ExternalInput").ap()
    counts = nc.dram_tensor("counts", [P, CT], dtf, kind="ExternalInput").ap()
    center_in = nc.dram_tensor("center_in", [P, CT * F], dtf,
                               kind="ExternalInput").ap()
    center_out = nc.dram_tensor("center_out", [P, CT * F], dtf,
                                kind="ExternalOutput").ap()
    lossp = nc.dram_tensor("lossp", [1, 4], dtf, kind="ExternalOutput").ap()
    aps = (h_blk, d_all, counts, center_in, center_out, lossp)

    with tile.TileContext(nc) as tc:
        with (
            tc.tile_pool(name="const", bufs=1) as cpool,
            tc.tile_pool(name="hp", bufs=4) as hpool,
            tc.tile_pool(name="ohp", bufs=4) as ohpool,
            tc.tile_pool(name="cls", bufs=4) as clspool,
            tc.tile_pool(name="scrp", bufs=3) as scrpool,
            tc.tile_pool(name="accp", bufs=1) as accpool,
            tc.tile_pool(name="ps", bufs=2, space="PSUM") as pspool,
            tc.tile_pool(name="psl", bufs=1, space="PSUM") as pslpool,
        ):
            pools = (cpool, hpool, ohpool, clspool, scrpool, accpool,
                     pspool, pslpool)
            if reps == 1:
                _emit_body(nc, tc, mybir, pools, n_j, aps, strip=strip)
            else:
                with tc.For_i(0, reps, 1):
                    _emit_body(nc, tc, mybir, pools, n_j, aps, strip=strip)
    nc.compile()
    _prog_cache[key] = nc
    return nc


def _unshard(results):
    parts = []
    for c in range(N_CORES):
        pm = results[c]["center_out"].reshape(P, CT, F).transpose(1, 0, 2)
        parts.append(pm.reshape(C_PAD, F)[:C_SHARD])
    new_center = np.concatenate(parts, axis=0)
    tot = np.zeros(3, np.float64)
    for c in range(N_CORES):
        lp = results[c]["lossp"][0]
        tot += lp[:3].astype(np.float64)
    loss = (tot[0] - 2.0 * tot[1] + tot[2]) / (BATCH * F)
    return np.asarray(loss, dtype=np.float32), new_center


def kernel(h, d, center):
    from concourse.bass_utils import run_bass_kernel_spmd

    in_maps, n_j = _shard_inputs(h, d, center)
    nc = build_program(n_j, reps=1)
    res = run_bass_kernel_spmd(nc, in_maps, core_ids=list(range(N_CORES)))
    return _unshard(res.results)


# revision 17
# speedup vs baseline: 5.0697x; 1.0677x over previous
"""CenterLoss kernel for 8 Trainium2 NeuronCores (Bass/Tile).

Problem (hardcoded, from nn_CenterLoss):
    h      [262144, 256] f32
    d      [262144]      int   (class ids in [0, 100000))
    center [100000, 256] f32
    returns (loss, new_center):
        loss       = mean((h - center[d])**2)                       scalar f32
        new_center = center + 0.1 * where(count>0, segmean(h)-center, 0)

Sharding strategy: shard along n_class. Core c owns classes
[c*12500, (c+1)*12500) and the matching center rows; the host routes each
batch row to the core owning its class (sorted by class id). Segment sums
then never need a cross-device reduction.

Device algorithm per core:
  - Classes are processed in 98 tiles of 128. For class tile j the batch
    rows belonging to it (padded with zero rows to n_j*128, n_j static and
    identical on every core so the SPMD program is uniform) are multiplied
    by an on-device-built one-hot selector and accumulated into PSUM:
        sums[128, 256] += onehot[128b, 128c].T @ h[128b, 256]
    The selector for all n_j sub-tiles is built with ONE broadcast
    is_equal compare against a resident iota tile (host pre-shifts the
    class ids by -128j so the compare needs no per-tile offset).
  - loss uses the expansion sum|h|^2 - 2*sum_k s_k.c_k + sum_k n_k|c_k|^2
    so center[d] is never gathered. The dot products are fused
    multiply+free-dim-reduce ops (scalar_tensor_tensor accum_out / ACT
    Square accum_out) over wide group tiles.
  - new_center tile = ainv*sums + oma*c: oma*c on the scalar engine
    (per-partition scale), then one fused stt op on the vector engine.
    ainv = alpha*present/count, oma = 1 - alpha*present per class.
  - All DRAM streams are laid out partition-major on the host so each
    multi-tile group moves with a single fully-contiguous DMA.
"""

import numpy as np

BATCH = 262144
F = 256
C = 100000
ALPHA = 0.1
N_CORES = 8
P = 128
C_SHARD = C // N_CORES          # 12500 classes per core
CT = (C_SHARD + P - 1) // P     # 98 class tiles per core
C_PAD = CT * P                  # 12544 rows incl. padding

H_GROUP_COLS = 12               # max batch sub-tiles per h DMA group
C_GROUP = 3                     # class tiles per PSUM group
CIN_PSGROUPS = 3                # PSUM groups per center DMA group
PSB = 512                       # PSUM bank stride (f32) per class tile
ACT_HSQ_MOD = 1                 # h-group idx % MOD == 0 -> |h|^2 on ACT
T2_ENGINE = "act"               # 'pool' | 'act': engine for oma*c

_prog_cache = {}


def _h_groups(n_j):
    """Greedily group consecutive class tiles so each h DMA moves at most
    H_GROUP_COLS 128-row sub-tiles."""
    groups = []
    cur = []
    cols = 0
    for j, nj in enumerate(n_j):
        if cur and cols + nj > H_GROUP_COLS:
            groups.append(cur)
            cur, cols = [], 0
        cur.append(j)
        cols += nj
    if cur:
        groups.append(cur)
    return groups


def _c_groups():
    return [list(range(g, min(g + C_GROUP, CT))) for g in range(0, CT, C_GROUP)]


def _shard_inputs(h, d, center):
    """Route batch rows to the core owning their class; build per-core
    partition-major blocks so every device DMA is contiguous.

    Returns (in_maps, n_j)."""
    h = np.ascontiguousarray(np.asarray(h, dtype=np.float32))
    d = np.asarray(d).astype(np.int64)
    center = np.asarray(center, dtype=np.float32)

    order = np.argsort(d, kind="stable")
    ds = d[order]
    core_bounds = np.searchsorted(ds, np.arange(N_CORES + 1) * C_SHARD)
    counts_global = np.bincount(d, minlength=C)

    tile_rows_all = np.zeros((N_CORES, CT), dtype=np.int64)
    cnt_pads = []
    for c in range(N_CORES):
        cnt_pad = np.zeros(C_PAD, np.int64)
        cnt_pad[:C_SHARD] = counts_global[c * C_SHARD:(c + 1) * C_SHARD]
        cnt_pads.append(cnt_pad)
        tile_rows_all[c] = cnt_pad.reshape(CT, P).sum(1)

    n_j = np.maximum(np.ceil(tile_rows_all / P).astype(np.int64).max(0), 1)
    nbt = int(n_j.sum())
    h_groups = _h_groups(n_j)

    in_maps = []
    for c in range(N_CORES):
        lo, hi = core_bounds[c], core_bounds[c + 1]
        rows_idx = order[lo:hi]
        dl = (ds[lo:hi] - c * C_SHARD).astype(np.float32)
        tile_rows = tile_rows_all[c]
        tb = np.concatenate([[0], np.cumsum(tile_rows)])
        hs = h[rows_idx]  # this core's rows, class-sorted

        # per-class-tile pieces, partition-major [P, n_j, F] / [P, n_j]
        # d is pre-shifted by -128*j so the device compare is vs iota 0..127
        pieces_h, pieces_d = [], []
        for j in range(CT):
            r0, r1 = tb[j], tb[j + 1]
            nr = r1 - r0
            njj = int(n_j[j])
            bh = np.zeros((njj * P, F), np.float32)
            bd = np.zeros((njj * P,), np.float32)
            bh[:nr] = hs[r0:r1]
            bd[:nr] = dl[r0:r1] - j * P
            pieces_h.append(bh.reshape(njj, P, F).transpose(1, 0, 2))
            pieces_d.append(bd.reshape(njj, P).T)

        # h: concatenated per h-group, each group contiguous [P, cols*F]
        h_parts = []
        for g in h_groups:
            grp = np.concatenate([pieces_h[j] for j in g], axis=1)  # [P,cols,F]
            h_parts.append(grp.reshape(P, -1))
        h_blk = np.concatenate([p.reshape(-1) for p in h_parts])

        d_all = np.concatenate(pieces_d, axis=1)  # [P, nbt]

        counts_pm = cnt_pads[c].astype(np.float32).reshape(CT, P).T  # [P, CT]

        cshard = np.zeros((C_PAD, F), np.float32)
        cshard[:C_SHARD] = center[c * C_SHARD:(c + 1) * C_SHARD]
        # partition-major grouped center: [P, CT*F], tile t cols [t*F,(t+1)*F]
        center_pm = cshard.reshape(CT, P, F).transpose(1, 0, 2).reshape(P, CT * F)

        in_maps.append({
            "h_blk": np.ascontiguousarray(h_blk),
            "d_all": np.ascontiguousarray(d_all),
            "counts": np.ascontiguousarray(counts_pm),
            "center_in": np.ascontiguousarray(center_pm),
        })
    return in_maps, tuple(int(x) for x in n_j)


def _emit_body(nc, tc, mybir, pools, n_j, aps, strip=0):
    """One full pass: segment sums + center update + loss partials."""
    dtf = mybir.dt.float32
    alu = mybir.AluOpType
    h_blk, d_all_d, counts, center_in, center_out, lossp = aps
    cpool, hpool, ohpool, clspool, scrpool, accpool, pspool, pslpool = pools

    nbt = int(sum(n_j))
    h_groups = _h_groups(n_j)
    c_groups = _c_groups()
    n_hg = len(h_groups)
    n_cg = len(c_groups)
    col_of = np.concatenate([[0], np.cumsum(n_j)]).astype(int)
    SCR_COLS = max(H_GROUP_COLS, C_GROUP) * F
    MAX_NJ = int(max(n_j))

    iota_i = cpool.tile([P, P], mybir.dt.int32, tag="iota_i")
    nc.gpsimd.iota(iota_i[:], pattern=[[1, P]], base=0, channel_multiplier=0)
    iota_f = cpool.tile([P, P], dtf, tag="iota_f")
    nc.vector.tensor_copy(iota_f[:], iota_i[:])
    ones = cpool.tile([P, 1], dtf, tag="ones")
    nc.vector.memset(ones[:], 1.0)

    # resident tables: pre-shifted class ids + per-class factors
    d_all = accpool.tile([P, nbt], dtf, tag="d_all")
    nc.sync.dma_start(d_all[:], d_all_d[:])
    cnt_all = accpool.tile([P, CT], dtf, tag="cnt_all")
    nc.sync.dma_start(cnt_all[:], counts[:])
    safe_all = accpool.tile([P, CT], dtf, tag="safe_all")
    nc.vector.tensor_scalar_max(safe_all[:], cnt_all[:], 1.0)
    inv_all = accpool.tile([P, CT], dtf, tag="inv_all")
    nc.vector.reciprocal(inv_all[:], safe_all[:])
    pres_all = accpool.tile([P, CT], dtf, tag="pres_all")  # 0/1 presence
    nc.vector.tensor_scalar(out=pres_all[:], in0=cnt_all[:], scalar1=0.5,
                            scalar2=None, op0=alu.is_ge)
    ainv_all = accpool.tile([P, CT], dtf, tag="ainv_all")  # alpha*present/cnt
    nc.vector.tensor_tensor(out=ainv_all[:], in0=pres_all[:], in1=inv_all[:],
                            op=alu.mult)
    nc.vector.tensor_scalar_mul(ainv_all[:], ainv_all[:], ALPHA)
    oma_all = accpool.tile([P, CT], dtf, tag="oma_all")    # 1 - alpha*present
    nc.vector.tensor_scalar(out=oma_all[:], in0=pres_all[:], scalar1=-ALPHA,
                            scalar2=1.0, op0=alu.mult, op1=alu.add)

    # loss staging columns
    acc = accpool.tile([P, 4], dtf, tag="acc")
    nc.vector.memset(acc[:], 0.0)
    hsq_g = accpool.tile([P, n_hg], dtf, tag="hsq_g")
    cross_g = accpool.tile([P, n_cg], dtf, tag="cross_g")
    csq_all = accpool.tile([P, CT], dtf, tag="csq_all")

    hgi = iter(enumerate(h_groups))
    ht = None
    ht_cols = ht_base = 0
    ohb = None

    cin_groups = [c_groups[i:i + CIN_PSGROUPS]
                  for i in range(0, len(c_groups), CIN_PSGROUPS)]
    cg = 0
    for cgrp in cin_groups:
        jc0 = cgrp[0][0]
        ntot = sum(len(t) for t in cgrp)
        cin = clspool.tile([P, CIN_PSGROUPS * C_GROUP * F], dtf, tag="cin")
        nc.sync.dma_start(cin[:, :ntot * F],
                          center_in[:, jc0 * F:(jc0 + ntot) * F])
        cout = clspool.tile([P, CIN_PSGROUPS * C_GROUP * F], dtf, tag="cout")

        for tiles in cgrp:
            ncls = len(tiles)
            j0 = tiles[0]
            psg = pspool.tile([P, C_GROUP * PSB], dtf, tag="psg")

            for t, j in enumerate(tiles):
                nj = n_j[j]
                col = col_of[j]
                if ht is None or col >= ht_base + ht_cols:
                    gidx, g = next(hgi)
                    cols = int(sum(n_j[jj] for jj in g))
                    ht = hpool.tile([P, H_GROUP_COLS * F], dtf, tag="ht")
                    nc.sync.dma_start(
                        ht[:, :cols * F],
                        h_blk[P * col_of[g[0]] * F:
                              P * (col_of[g[0]] + cols) * F].rearrange(
                            "(p x) -> p x", p=P))
                    ht_base, ht_cols = col, cols
                    if strip < 3 or strip == 5:
                        # one-hot selectors for ALL sub-tiles in the
                        # h-group, one broadcast compare
                        ohb = ohpool.tile([P, H_GROUP_COLS * P], dtf,
                                          tag="ohb")
                        nc.vector.tensor_tensor(
                            out=ohb[:, :cols * P],
                            in0=d_all[:, col:col + cols].to_broadcast(
                                [P, cols, P]),
                            in1=iota_f[:, None, :].broadcast_to(
                                [P, cols, P]),
                            op=alu.is_equal)
                    # sum_f h^2 over the whole group, one fused op
                    scr = scrpool.tile([P, SCR_COLS], dtf, tag="scr")
                    if strip >= 1:
                        pass
                    elif gidx % ACT_HSQ_MOD == 0:
                        nc.scalar.activation(
                            scr[:, :cols * F], ht[:, :cols * F],
                            mybir.ActivationFunctionType.Square,
                            accum_out=hsq_g[:, gidx:gidx + 1])
                    else:
                        nc.vector.scalar_tensor_tensor(
                            out=scr[:, :cols * F], in0=ht[:, :cols * F],
                            scalar=1.0, in1=ht[:, :cols * F],
                            op0=alu.mult, op1=alu.mult,
                            accum_out=hsq_g[:, gidx:gidx + 1])

                lc = col - ht_base
                if strip < 4 or strip == 5:
                    for s in range(nj):
                        nc.tensor.matmul(
                            psg[:, t * PSB:t * PSB + F],
                            lhsT=(ohb[:, (lc + s) * P:(lc + s + 1) * P]
                                  if (strip < 3 or strip == 5)
                                  else iota_f[:]),
                            rhs=ht[:, (lc + s) * F:(lc + s + 1) * F],
                            start=(s == 0), stop=(s == nj - 1))

                # new_c = ainv*sums + oma*c (oma*c on ACT, fused stt on DVE)
                tc_ = j - jc0
                ctsl = cin[:, tc_ * F:(tc_ + 1) * F]
                if strip < 2 or strip == 5:
                    if strip == 5:
                        t2 = ctsl  # timing probe: skip the oma pre-scale
                    else:
                        t2 = clspool.tile([P, F], dtf, tag="t2")
                        if T2_ENGINE == "pool":
                            nc.gpsimd.tensor_scalar_mul(
                                t2[:], ctsl, oma_all[:, j:j + 1])
                        else:
                            nc.scalar.activation(
                                t2[:], ctsl,
                                mybir.ActivationFunctionType.Copy,
                                scale=oma_all[:, j:j + 1])
                        t2 = t2[:]
                    nc.vector.scalar_tensor_tensor(
                        out=cout[:, tc_ * F:(tc_ + 1) * F],
                        in0=psg[:, t * PSB:t * PSB + F],
                        scalar=ainv_all[:, j:j + 1], in1=t2,
                        op0=alu.mult, op1=alu.add)
                if strip < 1:
                    # csq_all[:,j] = sum_f c^2 (ACT fused)
                    csq = clspool.tile([P, F], dtf, tag="csq")
                    nc.scalar.activation(
                        csq[:], ctsl, mybir.ActivationFunctionType.Square,
                        accum_out=csq_all[:, j:j + 1])

            if strip < 1:
                # cross term for the whole PSUM group, one fused op
                tg0 = (j0 - jc0) * F
                scr2 = scrpool.tile([P, SCR_COLS], dtf, tag="scr")
                psg_v = psg[:].rearrange("p (a b) -> p a b", b=PSB)[:, :ncls, :F]
                nc.vector.scalar_tensor_tensor(
                    out=scr2[:, :ncls * F].rearrange("p (a f) -> p a f", f=F),
                    in0=psg_v, scalar=1.0,
                    in1=cin[:, tg0:tg0 + ncls * F].rearrange(
                        "p (a f) -> p a f", f=F),
                    op0=alu.mult, op1=alu.mult,
                    accum_out=cross_g[:, cg:cg + 1])
            cg += 1

        nc.sync.dma_start(
            center_out[:, jc0 * F:(jc0 + ntot) * F],
            (cout if (strip < 2 or strip == 5) else cin)[:, :ntot * F])

    # fold staging: acc[:,0]=sum h^2, acc[:,1]=sum s.c, acc[:,2]=sum n|c|^2
    if strip >= 1:
        lt = accpool.tile([1, 4], dtf, tag="lt")
        nc.vector.memset(lt[:], 0.0)
        nc.sync.dma_start(lossp[:], lt[:])
        return
    nc.vector.tensor_reduce(acc[:, 0:1], hsq_g[:],
                            axis=mybir.AxisListType.X, op=alu.add)
    nc.vector.tensor_reduce(acc[:, 1:2], cross_g[:],
                            axis=mybir.AxisListType.X, op=alu.add)
    wc_all = accpool.tile([P, CT], dtf, tag="wc_all")
    nc.vector.tensor_tensor(out=wc_all[:], in0=csq_all[:], in1=cnt_all[:],
                            op=alu.mult)
    nc.vector.tensor_reduce(acc[:, 2:3], wc_all[:], axis=mybir.AxisListType.X,
                            op=alu.add)

    # reduce partition dim: lossp[1,4] = ones.T @ acc
    psl = pslpool.tile([1, 4], dtf, tag="psl")
    nc.tensor.matmul(psl[:], lhsT=ones[:], rhs=acc[:], start=True, stop=True)
    lt = accpool.tile([1, 4], dtf, tag="lt")
    nc.vector.tensor_copy(lt[:], psl[:])
    nc.sync.dma_start(lossp[:], lt[:])


def build_program(n_j, reps=1, strip=0):
    """Compile the SPMD program for the given static per-class-tile batch
    sub-tile counts. reps>1 wraps the body in a hardware loop (timing)."""
    key = (tuple(n_j), reps, strip)
    if key in _prog_cache:
        return _prog_cache[key]
    import concourse.bacc as bacc
    import concourse.mybir as mybir
    import concourse.tile as tile

    nbt = int(sum(n_j))
    dtf = mybir.dt.float32
    nc = bacc.Bacc("TRN2", target_bir_lowering=False, debug=False,
                   num_devices=N_CORES)
    h_blk = nc.dram_tensor("h_blk", [P * nbt * F], dtf,
                           kind="ExternalInput").ap()
    d_all = nc.dram_tensor("d_all", [P, nbt], dtf, kind="ExternalInput").ap()
    counts = nc.dram_tensor("counts", [P, CT], dtf, kind="ExternalInput").ap()
    center_in = nc.dram_tensor("center_in", [P, CT * F], dtf,
                               kind="ExternalInput").ap()
    center_out = nc.dram_tensor("center_out", [P, CT * F], dtf,
                                kind="ExternalOutput").ap()
    lossp = nc.dram_tensor("lossp", [1, 4], dtf, kind="ExternalOutput").ap()
    aps = (h_blk, d_all, counts, center_in, center_out, lossp)

    with tile.TileContext(nc) as tc:
        with (
            tc.tile_pool(name="const", bufs=1) as cpool,
            tc.tile_pool(name="hp", bufs=4) as hpool,
            tc.tile_pool(name="ohp", bufs=4) as ohpool,
            tc.tile_pool(name="cls", bufs=4) as clspool,
            tc.tile_pool(name="scrp", bufs=3) as scrpool,
            tc.tile_pool(name="accp", bufs=1) as accpool,
            tc.tile_pool(name="ps", bufs=2, space="PSUM") as pspool,
            tc.tile_pool(name="psl", bufs=1, space="PSUM") as pslpool,
        ):
            pools = (cpool, hpool, ohpool, clspool, scrpool, accpool,
                     pspool, pslpool)
            if reps == 1:
                _emit_body(nc, tc, mybir, pools, n_j, aps, strip=strip)
            else:
                with tc.For_i(0, reps, 1):
                    _emit_body(nc, tc, mybir, pools, n_j, aps, strip=strip)
    nc.compile()
    _prog_cache[key] = nc
    return nc


def _unshard(results):
    parts = []
    for c in range(N_CORES):
        pm = results[c]["center_out"].reshape(P, CT, F).transpose(1, 0, 2)
        parts.append(pm.reshape(C_PAD, F)[:C_SHARD])
    new_center = np.concatenate(parts, axis=0)
    tot = np.zeros(3, np.float64)
    for c in range(N_CORES):
        lp = results[c]["lossp"][0]
        tot += lp[:3].astype(np.float64)
    loss = (tot[0] - 2.0 * tot[1] + tot[2]) / (BATCH * F)
    return np.asarray(loss, dtype=np.float32), new_center


def kernel(h, d, center):
    from concourse.bass_utils import run_bass_kernel_spmd

    in_maps, n_j = _shard_inputs(h, d, center)
    nc = build_program(n_j, reps=1)
    last_err = None
    for _ in range(3):  # retry transient device/tunnel hiccups
        try:
            res = run_bass_kernel_spmd(nc, in_maps,
                                       core_ids=list(range(N_CORES)))
            return _unshard(res.results)
        except Exception as e:  # noqa: BLE001
            last_err = e
    raise last_err
